# revision 13
# baseline (speedup 1.0000x reference)
"""Causal self-attention (B=4, T=2048, C=1024, 16 heads) on 8 TRN2 NeuronCores.

Sharding: core = 2*b + g  (b = batch 0..3, g = head-group 0..1, 8 heads each).
Each core computes QKV for its 8 heads and causal attention.  The output
projection is computed as a PARTIAL sum over the core's local 512 channels
(full 1024 output columns), then a pair ReduceScatter(add) combines the two
halves and leaves each rank its own 512 output columns -- the collective is
pure DMA/CC work, fully off the PE critical path.

v3 schedule: single global PE stream, j-major (q-chunk outer, head-pair
inner) so each q-chunk's projection + ReduceScatter fires as soon as its 4
pair-chunks finish, spreading the collectives uniformly instead of
clustering them at the end.  Attention slots (per-head S matmuls -> exp on
ACT -> PV pair, LAG=2) interleave with deadline-paced fills (QKV m-tiles,
V t-tiles, projection psum chains).  The softmax reciprocal runs on DVE
(IEEE 1/x) and is broadcast across partitions by GPSIMD partition_broadcast
-- ACT does only exp, and the denominator path needs no PSUM bank, giving
the projection a dedicated bank (PSUM: 3 S + 4 y + 1 proj = 8).

Layouts (bf16 on device except f32 psum/yu/denominators; out f32):
  xt  = X[b]^T              [1024, 2048]   (host pre-transpose)
  Q^T, K^T                  [512, 2048]    hd on partitions (4 tiles, head pair each)
  V natural + ones column   [128, 8, 65] per k-tile (PV -> y^T and softmax denom)
  S^T = K^T.T @ Q^T         [k=128, q<=512] per head (1 psum bank each)
  P = exp(S^T) (bf16)       diag blocks masked by 0/1 triangle multiply (DVE)
  y^T[65, q] = V_aug^T @ P  row 64 = softmax denominator
  proj (per q-chunk j, m-tile, col-half): po = sum_p yt[p]^T @ wp[p] (psum
    chain over the 4 LOCAL pairs only) -> rs_in[j] -> ReduceScatter(add)
    writes out[512j:512j+512, :] directly.
"""

import numpy as np
import ml_dtypes

B, T, C = 4, 2048, 1024
H, HD = 16, 64
NCORES = 8
HL = 8            # local heads per core
NP = 4            # head pairs per core
NKT = T // 128    # 16 k-tiles
NJ = 4            # q-chunks of 512
RG = [[0, 1], [2, 3], [4, 5], [6, 7]]
LAG = 2
ND = 5            # norm_b delay in slots after norm_a

_cache = {}


def _split_multiwait(nc):
    """walrus in this image accepts only ONE embedded wait per instruction;
    split extras into single-wait NoOps on the same engine just before it."""
    import concourse.mybir as mybir

    for fn in nc.m.functions:
        for blk in fn.blocks:
            new = []
            for inst in blk.instructions:
                si = getattr(inst, "sync_info", None)
                if si is not None and si.on_wait is not None and len(si.on_wait) > 1:
                    waits = list(si.on_wait)
                    for k, w in enumerate(waits[:-1]):
                        nop = mybir.InstNoOp(name=f"{inst.name}-w{k}")
                        nop.engine = inst.engine
                        nop.sync_info = mybir.SyncInfo(on_wait=[w], on_update=[])
                        new.append(nop)
                    inst.sync_info = mybir.SyncInfo(
                        on_wait=[waits[-1]], on_update=list(si.on_update or [])
                    )
                new.append(inst)
            blk.instructions = new


def _build(split_waits=True):
    import heapq
    import concourse.bass as bass
    import concourse.mybir as mybir
    import concourse.tile as tile
    from contextlib import ExitStack

    bf16 = mybir.dt.bfloat16
    f32 = mybir.dt.float32
    AF = mybir.ActivationFunctionType
    nc = bass.Bass(num_devices=NCORES)

    xt = nc.declare_dram_parameter("xt", [C, T], bf16, isOutput=False)
    wqk = nc.declare_dram_parameter("wqk", [C, 1024], bf16, isOutput=False)
    wv = nc.declare_dram_parameter("wv", [C, 512], bf16, isOutput=False)
    wp = nc.declare_dram_parameter("wp", [512, 1024], bf16, isOutput=False)
    out = nc.declare_dram_parameter("out", [T, 512], f32, isOutput=True)

    rs_in = [nc.dram_tensor(f"rs_in{j}", [2, 512, 512], f32) for j in range(NJ)]
    rs_out = [
        nc.dram_tensor(f"rs_out{j}", [512, 512], f32) for j in range(NJ)
    ]

    with ExitStack() as ctx:
        tc = ctx.enter_context(tile.TileContext(nc))
        pers = ctx.enter_context(tc.tile_pool(name="pers", bufs=1))
        pp = ctx.enter_context(tc.tile_pool(name="pp", bufs=4))
        yup = ctx.enter_context(tc.tile_pool(name="yup", bufs=4))
        osp = ctx.enter_context(tc.tile_pool(name="osp", bufs=2))
        spp = ctx.enter_context(tc.tile_pool(name="spp", bufs=3, space="PSUM"))
        ypp = ctx.enter_context(tc.tile_pool(name="ypp", bufs=4, space="PSUM"))
        php = ctx.enter_context(tc.tile_pool(name="php", bufs=1, space="PSUM"))

        # ---------------- persistent tiles ----------------
        xt_sb = [pers.tile([128, T], bf16, tag=f"xt{i}", name=f"xt{i}") for i in range(8)]
        wqk_sb = [pers.tile([128, 1024], bf16, tag=f"wqk{i}", name=f"wqk{i}") for i in range(8)]
        wv_sb = [pers.tile([128, 512], bf16, tag=f"wv{i}", name=f"wv{i}") for i in range(8)]
        wp_sb = [pers.tile([128, 1024], bf16, tag=f"wp{i}", name=f"wp{i}") for i in range(4)]
        qt_sb = [pers.tile([128, T], bf16, tag=f"qt{p}", name=f"qt{p}") for p in range(NP)]
        kt_sb = [pers.tile([128, T], bf16, tag=f"kt{p}", name=f"kt{p}") for p in range(NP)]
        v_sb = [pers.tile([128, HL, 65], bf16, tag=f"v{i}", name=f"v{i}") for i in range(NKT)]
        yt_sb = [pers.tile([128, T], bf16, tag=f"yt{p}", name=f"yt{p}") for p in range(NP)]
        # 0/1 causal keep-mask for the two heads' diagonal 128x128 blocks
        mask2 = pers.tile([128, 2, 128], bf16, tag="mask2", name="mask2")
        # selection matrix for the K=33 denominator-broadcast matmul:
        # db[m, q] = sum_r sel[r, m] * dvb[r, q]; row 0 selects head a
        # (m < 64), row 32 head b (m >= 64); rows 1..31 are zero and dvb's
        # rows 1..31 stay 0.0 (memset once) so they contribute nothing.
        sel_sb = pers.tile([33, 128], bf16, tag="sel", name="sel")
        dvb = pers.tile([33, 512], bf16, tag="dvb", name="dvb")

        # ---------------- input loads (priority-ordered, 3 queues) ----------
        # need order: wv + xt n0 (v tiles 0..3), wqk (qk pair fills), xt n1..3,
        # wp (projection, first needed ~group 1).  Queues: sync carries wv
        # then wqk then wp; gpsimd/scalar carry xt (even/odd kc row-tiles).
        for i in range(8):
            nc.sync.dma_start(out=wv_sb[i], in_=wv[128 * i : 128 * i + 128, :])
        for n in range(NJ):
            for i in range(8):
                eng = nc.gpsimd if i % 2 == 0 else nc.scalar
                eng.dma_start(
                    out=xt_sb[i][:, 512 * n : 512 * n + 512],
                    in_=xt[128 * i : 128 * i + 128, 512 * n : 512 * n + 512],
                )
            if n == 0:
                for i in range(8):
                    nc.sync.dma_start(
                        out=wqk_sb[i], in_=wqk[128 * i : 128 * i + 128, :]
                    )
        for i in range(4):
            nc.sync.dma_start(out=wp_sb[i], in_=wp[128 * i : 128 * i + 128, :])

        # masks (after the DMA issues so they don't delay the loads):
        # mask2[r, s, q'] = 1 iff q' >= r (keep), else 0
        nc.gpsimd.memset(mask2, 1.0)
        nc.gpsimd.affine_select(
            out=mask2, in_=mask2, compare_op=mybir.AluOpType.is_ge, fill=0.0,
            base=0, pattern=[[0, 2], [1, 128]], channel_multiplier=-1,
        )
        nc.vector.memset(sel_sb, 0.0)
        nc.vector.memset(sel_sb[0:1, 0:64], 1.0)
        nc.vector.memset(sel_sb[32:33, 64:128], 1.0)
        nc.vector.memset(dvb, 0.0)

        # ---------------- fill closures ----------------
        def v_ttile(i):
            def run():
                ps = ypp.tile([128, 512], f32, tag="y", name="psv")
                for kc in range(8):
                    nc.tensor.matmul(
                        ps,
                        lhsT=xt_sb[kc][:, 128 * i : 128 * i + 128],
                        rhs=wv_sb[kc],
                        start=(kc == 0), stop=(kc == 7),
                    )
                nc.vector.tensor_copy(
                    out=v_sb[i][:, :, 0:64],
                    in_=ps.rearrange("p (h d) -> p h d", h=HL),
                )
                nc.vector.memset(v_sb[i][:, :, 64:65], 1.0)
            return run

        def qk_mtile(p, which, n):
            col0 = 128 * p + (512 if which == 1 else 0)
            dst = kt_sb[p] if which == 1 else qt_sb[p]

            def run():
                ps = ypp.tile([128, 512], f32, tag="y", name="psqk")
                for kc in range(8):
                    nc.tensor.matmul(
                        ps,
                        lhsT=wqk_sb[kc][:, col0 : col0 + 128],
                        rhs=xt_sb[kc][:, 512 * n : 512 * n + 512],
                        start=(kc == 0), stop=(kc == 7),
                    )
                nc.vector.tensor_copy(
                    out=dst[:, 512 * n : 512 * n + 512], in_=ps,
                )
            return run

        def proj_mh(j, m, h, last):
            """Partial projection for out rows 128m..128m+128, cols
            512h..512h+512: psum chain over the 4 LOCAL pairs' yt.  After the
            last half-tile of chunk j, trigger the pair ReduceScatter(add)
            straight into out[512j:512j+512, :]."""
            def run():
                po = php.tile([128, 512], f32, tag="po", name="po")
                for p in range(NP):
                    nc.tensor.matmul(
                        po,
                        lhsT=yt_sb[p][:, 128 * m : 128 * m + 128],
                        rhs=wp_sb[p][:, 512 * h : 512 * h + 512],
                        start=(p == 0), stop=(p == NP - 1),
                    )
                os = osp.tile([128, 512], f32, tag="os", name="os")
                nc.vector.tensor_copy(os, po)
                r0 = 128 * (m - 4 * j)
                nc.sync.dma_start(out=rs_in[j][h, r0 : r0 + 128, :], in_=os)
                if last:
                    nc.gpsimd.collective_compute(
                        "ReduceScatter", mybir.AluOpType.add, replica_groups=RG,
                        ins=[rs_in[j].ap().opt()],
                        outs=[rs_out[j].ap().opt()],
                    )
                    nc.sync.dma_start(
                        out=out[512 * j : 512 * j + 512, :], in_=rs_out[j][:, :]
                    )
            return run

        # ---------------- attention slot pieces ----------------
        def emit_s(p, j, i):
            """S^T matmuls (one per head, K=64) + exp -> P tile; diag
            blocks masked by 0/1 triangle multiply on DVE."""
            dlt = 128 * i - 512 * j
            de = max(0, dlt)
            diag = dlt >= 0
            pt = pp.tile([128, 1024], bf16, tag="pt", name="pt")
            for h in range(2):  # head pair, separate 1-bank psum tiles
                r0, c0 = 64 * h, 512 * h
                sp = spp.tile([128, 512], f32, tag="sp", name="sp")
                nc.tensor.matmul(
                    sp[:, de:512],
                    lhsT=kt_sb[p][r0 : r0 + 64, 128 * i : 128 * i + 128],
                    rhs=qt_sb[p][r0 : r0 + 64, 512 * j + de : 512 * j + 512],
                    start=True, stop=True,
                )
                nc.scalar.activation(
                    pt[:, c0 + de : c0 + 512], sp[:, de:512], AF.Exp
                )
            if diag:
                ptm = pt.rearrange("p (s q) -> p s q", s=2)[:, :, de : de + 128]
                nc.vector.tensor_mul(ptm, ptm, mask2)
            return pt

        def emit_pv(p, j, i, pt, ya, yb, nkt):
            dlt = 128 * i - 512 * j
            de = max(0, dlt)
            nc.tensor.matmul(
                ya[0:65, de:512],
                lhsT=v_sb[i][:, 2 * p, :], rhs=pt[:, de:512],
                start=(i == 0), stop=(i == nkt - 1), skip_group_check=True,
            )
            nc.tensor.matmul(
                yb[0:65, de:512],
                lhsT=v_sb[i][:, 2 * p + 1, :], rhs=pt[:, 512 + de : 1024],
                start=(i == 0), stop=(i == nkt - 1), skip_group_check=True,
            )

        # norm part A (immediate after last PV): evacuate y, reciprocal on DVE
        def make_norm_a(p, j, ya, yb):
            state = {}

            def run():
                yu_a = yup.tile([65, 512], f32, tag="yu", name="yu")
                nc.vector.tensor_copy(yu_a, ya[0:65, :])
                yu_b = yup.tile([65, 512], f32, tag="yu", name="yu")
                nc.vector.tensor_copy(yu_b, yb[0:65, :])
                # IEEE 1/x on the denominator rows, in place (partition 64 is
                # 32-aligned for the DVE write)
                nc.vector.reciprocal(out=yu_a[64:65, :], in_=yu_a[64:65, :])
                nc.vector.reciprocal(out=yu_b[64:65, :], in_=yu_b[64:65, :])
                state["t"] = (yu_a, yu_b)
            return state, run

        # norm part B (delayed ~ND slots): stage both reciprocal rows into
        # the persistent dvb (rows 0 / 32; rows 1..31 stay 0.0 against the
        # zero rows of sel), broadcast across partitions via one K=33 PE
        # matmul, normalize into yt on DVE.
        def make_norm_b(p, j, state):
            def run():
                yu_a, yu_b = state["t"]
                cs = slice(512 * j, 512 * j + 512)
                with nc.allow_low_precision(reason="softmax denom bf16"):
                    nc.vector.tensor_copy(dvb[0:1, :], yu_a[64:65, :])
                    nc.vector.tensor_copy(dvb[32:33, :], yu_b[64:65, :])
                db = ypp.tile([128, 512], f32, tag="y", name="db")
                nc.tensor.matmul(
                    db, lhsT=sel_sb, rhs=dvb,
                    start=True, stop=True, skip_group_check=True,
                )
                with nc.allow_low_precision(reason="y normalize bf16"):
                    nc.vector.tensor_mul(
                        yt_sb[p][0:64, cs], yu_a[0:64, :], db[0:64, :]
                    )
                    nc.vector.tensor_mul(
                        yt_sb[p][64:128, cs], yu_b[0:64, :], db[64:128, :]
                    )
            return run

        # ---------------- global slot stream (j-major) ----------------
        chunks = [(p, j) for j in range(NJ) for p in range(NP)]
        slots = []
        base = {}
        for p, j in chunks:
            base[(p, j)] = len(slots)
            for i in range(4 * j + 4):
                slots.append((p, j, i))
        n_s = len(slots)
        base_g = [base[(0, j)] for j in range(NJ)]         # group starts
        glen = [4 * (4 * j + 4) for j in range(NJ)]        # group lengths

        # fill queue: (deadline_slot, seq, closure)
        fills = []
        seq = [0]

        def push_fill(deadline, run):
            heapq.heappush(fills, (deadline, seq[0], run))
            seq[0] += 1

        # v tiles 4..15 during group (i//4 - 1): spread over its second half
        for i in range(4, 16):
            gi = i // 4
            dl = base_g[gi - 1] + (glen[gi - 1] * (5 + i % 4)) // 10
            push_fill(min(dl, base_g[gi] + i - 2), v_ttile(i))
        # qk(p, *, n): group 0 pairs 1..3 just before their chunks; groups
        # n>=1 spread over the first half of group n-1
        for p in range(1, NP):
            push_fill(4 * p - 3, qk_mtile(p, 0, 0))
            push_fill(4 * p - 2, qk_mtile(p, 1, 0))
        for n in range(1, NJ):
            for p in range(NP):
                dl = base_g[n - 1] + (glen[n - 1] * (1 + 2 * p)) // 20
                push_fill(dl, qk_mtile(p, 0, n))
                push_fill(dl + 1, qk_mtile(p, 1, n))

        # pre-stream: q/k (p0, n0) + v0..3
        qk_mtile(0, 0, 0)()
        qk_mtile(0, 1, 0)()
        for i in range(4):
            v_ttile(i)()

        ypss = {}           # chunkidx -> (ya, yb)
        pts = {}            # slot t -> P tile
        normq = []          # [slots_remaining, norm_b closure]
        chunk_of_slot = {}
        for idx, (p, j) in enumerate(chunks):
            for i in range(4 * j + 4):
                chunk_of_slot[len(chunk_of_slot)] = idx

        for t in range(n_s + LAG):
            # trailing PV + immediate norm-a for finished chunks first so
            # their releases precede this slot's fills/S on the engine FIFOs
            tt = t - LAG
            if tt >= 0:
                p2, j2, i2 = slots[tt]
                idx2 = chunk_of_slot[tt]
                ya, yb = ypss[idx2]
                nkt2 = 4 * j2 + 4
                emit_pv(p2, j2, i2, pts.pop(tt), ya, yb, nkt2)
                if i2 == nkt2 - 1:
                    state, norm_a = make_norm_a(p2, j2, ya, yb)
                    norm_a()
                    normq.append([ND, make_norm_b(p2, j2, state)])
                    del ypss[idx2]
                    if p2 == NP - 1:
                        # group j2 complete: partial projection of its rows.
                        # Deadline after the last norm_b has run (~t+ND+2),
                        # spread into the next group's stretch; j=3 drains
                        # post-stream.
                        for k in range(8):
                            m, hh = 4 * j2 + k // 2, k % 2
                            if j2 < NJ - 1:
                                dl = max(
                                    t + ND + 2 + k,
                                    base_g[j2 + 1] + (glen[j2 + 1] * (2 + k)) // 24,
                                )
                            else:
                                dl = n_s + LAG + 1 + k
                            push_fill(dl, proj_mh(j2, m, hh, last=(k == 7)))
            # delayed norm-b closures count down in slots
            for ent in normq:
                ent[0] -= 1
            while normq and normq[0][0] <= 0:
                normq.pop(0)[1]()
            # emit due fills
            while fills and fills[0][0] <= t:
                heapq.heappop(fills)[2]()
            if t < n_s:
                p, j, i = slots[t]
                if i == 0:
                    ypss[chunk_of_slot[t]] = (
                        ypp.tile([128, 512], f32, tag="y", name="ya"),
                        ypp.tile([128, 512], f32, tag="y", name="yb"),
                    )
                pts[t] = emit_s(p, j, i)

        # drain: remaining norm-b and fills (proj of last q-chunk)
        for ent in normq:
            ent[1]()
        while fills:
            heapq.heappop(fills)[2]()

    if split_waits:
        _split_multiwait(nc)
    return nc


def _get_nc():
    if "nc" not in _cache:
        _cache["nc"] = _build()
    return _cache["nc"]


def _make_in_maps(x, w_attn, b_attn, w_proj, b_proj):
    bf = ml_dtypes.bfloat16
    in_maps = []
    for core in range(NCORES):
        b, g = core // 2, core % 2
        qs = slice(512 * g, 512 * g + 512)
        ks = slice(1024 + 512 * g, 1024 + 512 * g + 512)
        vs = slice(2048 + 512 * g, 2048 + 512 * g + 512)
        xt = np.ascontiguousarray(np.asarray(x[b]).T).astype(bf)
        wqk = np.concatenate(
            [np.asarray(w_attn[:, qs], dtype=np.float64) * 0.125,
             np.asarray(w_attn[:, ks], dtype=np.float64)], axis=1
        ).astype(bf)
        wv = np.asarray(w_attn[:, vs]).astype(bf)
        wp = np.asarray(w_proj[512 * g : 512 * g + 512, :]).astype(bf)
        in_maps.append(dict(xt=xt, wqk=wqk, wv=wv, wp=wp))
    return in_maps


def _run(in_maps, trace=False, **kw):
    from concourse.bass_utils import run_bass_kernel_spmd

    nc = _get_nc()
    return run_bass_kernel_spmd(
        nc, in_maps, core_ids=list(range(NCORES)), trace=trace, **kw
    )


def kernel(x, w_attn, b_attn, w_proj, b_proj):
    in_maps = _make_in_maps(x, w_attn, b_attn, w_proj, b_proj)
    res = _run(in_maps, trace=False)
    y = np.zeros((B, T, C), np.float32)
    for core in range(NCORES):
        b, g = core // 2, core % 2
        y[b][:, 512 * g : 512 * g + 512] = np.asarray(res.results[core]["out"])
    return y


# revision 25
# speedup vs baseline: 1.4077x; 1.4077x over previous
"""Causal self-attention (B=4, T=2048, C=1024, 16 heads) on 8 TRN2 NeuronCores.

Sharding: core = 2*b + g  (b = batch 0..3, g = head-group 0..1, 8 heads each).
Each core computes QKV for its 8 heads, causal attention, then the columns
out[:, 512g:512g+512] of the output projection.  The projection needs the
full y = concat(heads), so the two cores of each batch exchange their yT
chunks with a pair AllGather (bf16, partition-axis concat -> absolute head
order identical on both ranks -> uniform SPMD graph).

v4 schedule: single global PE stream, J-MAJOR (q-chunk outer, head-pair
inner).  Each q-chunk j's four pair-chunks finish back to back, their
AllGathers fire immediately (spread uniformly over the run instead of
clustering at the end), and the projection of rows [512j, 512j+512) runs
during group j+1 -- only the last group's projection remains after the
slot stream, so the tail is one AllGather + 4 m-tiles.

Attention slots (S-pair matmul -> one exp on ACT -> PV pair, LAG=2) are
interleaved with deadline-paced fills (QKV m-tiles, V t-tiles, projection
chains).  ACT does ONLY exp (one act per slot, 2-segment strided AP on
diagonal tiles).  The causal mask is a DVE multiply with a precomputed 0/1
triangle after exp.  The softmax reciprocal runs on DVE
(reciprocal_approx_fast, fp32, ~18 bits) and is broadcast across partitions
by a K=33 PE matmul against a 0/1 selection matrix.

Input DMA is priority-ordered across 4 queues (sync/vector carry wv then
wqk split even/odd then wp; gpsimd/scalar carry xt even/odd kc) so the
first V/QK tiles land as early as possible after the ~11us DMA-init dead
time.

Layouts (bf16 on device except f32 psum/yu; out f32):
  xt  = X[b]^T              [1024, 2048]   (host pre-transpose)
  Q^T, K^T                  [512, 2048]    hd on partitions (4 tiles, head pair each)
  V natural + ones column   [128, 8, 65] per k-tile (PV -> y^T and softmax denom)
  S^T = K^T.T @ Q^T         [k=128, q<=512] per head, pair packed via row groups
  P = exp(S^T) (bf16)       diag blocks masked by 0/1 triangle multiply
  y^T[65, q] = V_aug^T @ P  row 64 = softmax denominator
  proj (per q-chunk j, after pair AllGather of all 4 pairs):
    out[128m:128m+128, :] = sum_c ytf[c][:, m-block]^T @ wp[c]   (psum chain)
"""

import numpy as np
import ml_dtypes

B, T, C = 4, 2048, 1024
H, HD = 16, 64
NCORES = 8
HL = 8            # local heads per core
NP = 4            # head pairs per core
NKT = T // 128    # 16 k-tiles
NJ = 4            # q-chunks of 512
RG = [[0, 1], [2, 3], [4, 5], [6, 7]]
LAG = 2
ND = 5            # norm_b delay in slots after norm_a

_cache = {}


def _split_multiwait(nc):
    """walrus in this image accepts only ONE embedded wait per instruction;
    split extras into single-wait NoOps on the same engine just before it."""
    import concourse.mybir as mybir

    for fn in nc.m.functions:
        for blk in fn.blocks:
            new = []
            for inst in blk.instructions:
                si = getattr(inst, "sync_info", None)
                if si is not None and si.on_wait is not None and len(si.on_wait) > 1:
                    waits = list(si.on_wait)
                    for k, w in enumerate(waits[:-1]):
                        nop = mybir.InstNoOp(name=f"{inst.name}-w{k}")
                        nop.engine = inst.engine
                        nop.sync_info = mybir.SyncInfo(on_wait=[w], on_update=[])
                        new.append(nop)
                    inst.sync_info = mybir.SyncInfo(
                        on_wait=[waits[-1]], on_update=list(si.on_update or [])
                    )
                new.append(inst)
            blk.instructions = new


def _build(split_waits=True):
    import heapq
    import concourse.bass as bass
    import concourse.mybir as mybir
    import concourse.tile as tile
    from contextlib import ExitStack

    bf16 = mybir.dt.bfloat16
    f32 = mybir.dt.float32
    AF = mybir.ActivationFunctionType
    nc = bass.Bass(num_devices=NCORES)

    xt = nc.declare_dram_parameter("xt", [C, T], bf16, isOutput=False)
    wqk = nc.declare_dram_parameter("wqk", [C, 1024], bf16, isOutput=False)
    wv = nc.declare_dram_parameter("wv", [C, 512], bf16, isOutput=False)
    wp = nc.declare_dram_parameter("wp", [C, 512], bf16, isOutput=False)
    out = nc.declare_dram_parameter("out", [T, 512], f32, isOutput=True)

    ag_in = [[nc.dram_tensor(f"ag_in{p}_{j}", [128, 512], bf16) for j in range(NJ)]
             for p in range(NP)]
    ag_out = [[nc.dram_tensor(f"ag_out{p}_{j}", [256, 512], bf16) for j in range(NJ)]
              for p in range(NP)]

    with ExitStack() as ctx:
        tc = ctx.enter_context(tile.TileContext(nc))
        pers = ctx.enter_context(tc.tile_pool(name="pers", bufs=1))
        pp = ctx.enter_context(tc.tile_pool(name="pp", bufs=4))
        yup = ctx.enter_context(tc.tile_pool(name="yup", bufs=4))
        dvp = ctx.enter_context(tc.tile_pool(name="dvp", bufs=2))
        osp = ctx.enter_context(tc.tile_pool(name="osp", bufs=2))
        spp = ctx.enter_context(tc.tile_pool(name="spp", bufs=2, space="PSUM"))
        ypp = ctx.enter_context(tc.tile_pool(name="ypp", bufs=4, space="PSUM"))

        # ---------------- persistent tiles ----------------
        xt_sb = [pers.tile([128, T], bf16, tag=f"xt{i}", name=f"xt{i}") for i in range(8)]
        wqk_sb = [pers.tile([128, 1024], bf16, tag=f"wqk{i}", name=f"wqk{i}") for i in range(8)]
        wv_sb = [pers.tile([128, 512], bf16, tag=f"wv{i}", name=f"wv{i}") for i in range(8)]
        wp_sb = [pers.tile([128, 512], bf16, tag=f"wp{i}", name=f"wp{i}") for i in range(8)]
        qt_sb = [pers.tile([128, T], bf16, tag=f"qt{p}", name=f"qt{p}") for p in range(NP)]
        kt_sb = [pers.tile([128, T], bf16, tag=f"kt{p}", name=f"kt{p}") for p in range(NP)]
        v_sb = [pers.tile([128, HL, 65], bf16, tag=f"v{i}", name=f"v{i}") for i in range(NKT)]
        yt_sb = [pers.tile([128, T], bf16, tag=f"yt{p}", name=f"yt{p}") for p in range(NP)]
        ytf_sb = [pers.tile([128, T], bf16, tag=f"ytf{c}", name=f"ytf{c}") for c in range(8)]
        # 0/1 causal keep-mask for the two heads' diagonal 128x128 blocks
        mask2 = pers.tile([128, 2, 128], bf16, tag="mask2", name="mask2")
        # selection matrix for the single K=33 denominator-broadcast matmul:
        # db[m, q] = sum_r sel[r, m] * dvb[r, q]; row 0 selects head a
        # (m < 64), row 32 head b (m >= 64); rows 1..31 of both sel and dvb
        # are zero so they contribute nothing.
        sel_sb = pers.tile([33, 128], bf16, tag="sel", name="sel")
        dvb = pers.tile([33, 512], bf16, tag="dvb", name="dvb")
        # persistent Ln input; rows 1..31 stay 1.0 forever -> Ln = 0 ->
        # Exp(-0) = 1.0 (finite) in dvb's unused rows, zeroed by sel
        dv2 = pers.tile([33, 512], f32, tag="dv2", name="dv2")

        # ---------------- input loads (priority-ordered, 3 queues) ----------
        # sync: wqk pair-major (pair 0 first -> first QK m-tiles asap), wp.
        # scalar: wv odd rows, xt odd rows n-major.  gpsimd: wv even, xt even.
        # wqk host layout is PAIR-BLOCKED: cols [256p:256p+128]=q_p,
        # [256p+128:256p+256]=k_p, so per-pair loads are contiguous.
        for p in range(NP):
            for i in range(8):
                nc.sync.dma_start(
                    out=wqk_sb[i][:, 256 * p : 256 * p + 256],
                    in_=wqk[128 * i : 128 * i + 128, 256 * p : 256 * p + 256],
                )
        for i in range(8):
            eng = nc.gpsimd if i % 2 == 0 else nc.scalar
            eng.dma_start(out=wv_sb[i], in_=wv[128 * i : 128 * i + 128, :])
        for n in range(NJ):
            for i in range(8):
                eng = nc.gpsimd if i % 2 == 0 else nc.scalar
                eng.dma_start(
                    out=xt_sb[i][:, 512 * n : 512 * n + 512],
                    in_=xt[128 * i : 128 * i + 128, 512 * n : 512 * n + 512],
                )
        for i in range(8):
            nc.sync.dma_start(out=wp_sb[i], in_=wp[128 * i : 128 * i + 128, :])

        # masks (after the DMA issues so they don't delay the loads):
        # mask2[r, s, q'] = 1 iff q' >= r (keep), else 0
        nc.gpsimd.memset(mask2, 1.0)
        nc.gpsimd.affine_select(
            out=mask2, in_=mask2, compare_op=mybir.AluOpType.is_ge, fill=0.0,
            base=0, pattern=[[0, 2], [1, 128]], channel_multiplier=-1,
        )
        nc.vector.memset(sel_sb, 0.0)
        nc.vector.memset(sel_sb[0:1, 0:64], 1.0)
        nc.vector.memset(sel_sb[32:33, 64:128], 1.0)
        nc.vector.memset(dvb, 0.0)
        nc.vector.memset(dv2, 1.0)

        # ---------------- fill closures ----------------
        def v_ttile(i):
            def run():
                ps = ypp.tile([128, 512], f32, tag="y", name="psv")
                for kc in range(8):
                    nc.tensor.matmul(
                        ps,
                        lhsT=xt_sb[kc][:, 128 * i : 128 * i + 128],
                        rhs=wv_sb[kc],
                        start=(kc == 0), stop=(kc == 7),
                    )
                nc.vector.tensor_copy(
                    out=v_sb[i][:, :, 0:64],
                    in_=ps.rearrange("p (h d) -> p h d", h=HL),
                )
                nc.vector.memset(v_sb[i][:, :, 64:65], 1.0)
            return run

        def qk_mtile(p, which, n):
            col0 = 256 * p + 128 * which
            dst = kt_sb[p] if which == 1 else qt_sb[p]

            def run():
                ps = ypp.tile([128, 512], f32, tag="y", name="psqk")
                for kc in range(8):
                    nc.tensor.matmul(
                        ps,
                        lhsT=wqk_sb[kc][:, col0 : col0 + 128],
                        rhs=xt_sb[kc][:, 512 * n : 512 * n + 512],
                        start=(kc == 0), stop=(kc == 7),
                    )
                nc.vector.tensor_copy(
                    out=dst[:, 512 * n : 512 * n + 512], in_=ps,
                )
            return run

        def proj_m(m):
            def run():
                po = ypp.tile([128, 512], f32, tag="y", name="po")
                for c in range(8):
                    nc.tensor.matmul(
                        po,
                        lhsT=ytf_sb[c][:, 128 * m : 128 * m + 128],
                        rhs=wp_sb[c],
                        start=(c == 0), stop=(c == 7),
                    )
                os = osp.tile([128, 512], f32, tag="os", name="os")
                nc.vector.tensor_copy(os, po)
                nc.gpsimd.dma_start(out=out[128 * m : 128 * m + 128, :], in_=os)
            return run

        # ---------------- attention slot pieces ----------------
        def emit_s(p, j, i):
            """S^T pair matmuls (row-group packed) + exp -> P tile; diag
            blocks masked by 0/1 triangle multiply on DVE."""
            dlt = 128 * i - 512 * j
            de = max(0, dlt)
            diag = dlt >= 0
            sp = spp.tile([128, 1024], f32, tag="sp", name="sp")
            for h in range(2):  # head pair (K=64 each, packed on row groups)
                r0, c0 = 64 * h, 512 * h
                nc.tensor.matmul(
                    sp[:, c0 + de : c0 + 512],
                    lhsT=kt_sb[p][r0 : r0 + 64, 128 * i : 128 * i + 128],
                    rhs=qt_sb[p][r0 : r0 + 64, 512 * j + de : 512 * j + 512],
                    start=True, stop=True, skip_group_check=True,
                )
            pt = pp.tile([128, 1024], bf16, tag="pt", name="pt")
            if de > 0:
                sp3 = sp.rearrange("p (s q) -> p s q", s=2)[:, :, de:512]
                pt3 = pt.rearrange("p (s q) -> p s q", s=2)[:, :, de:512]
                nc.scalar.activation(pt3, sp3, AF.Exp)
            else:
                nc.scalar.activation(pt, sp, AF.Exp)
            if diag:
                ptm = pt.rearrange("p (s q) -> p s q", s=2)[:, :, de : de + 128]
                nc.vector.tensor_mul(ptm, ptm, mask2)
            return pt

        def emit_pv(p, j, i, pt, ya, yb, nkt):
            dlt = 128 * i - 512 * j
            de = max(0, dlt)
            nc.tensor.matmul(
                ya[0:65, de:512],
                lhsT=v_sb[i][:, 2 * p, :], rhs=pt[:, de:512],
                start=(i == 0), stop=(i == nkt - 1), skip_group_check=True,
            )
            nc.tensor.matmul(
                yb[0:65, de:512],
                lhsT=v_sb[i][:, 2 * p + 1, :], rhs=pt[:, 512 + de : 1024],
                start=(i == 0), stop=(i == nkt - 1), skip_group_check=True,
            )

        # norm part A (immediate after last PV): evacuate y to SBUF
        def make_norm_a(p, j, ya, yb):
            state = {}

            def run():
                yu_a = yup.tile([65, 512], f32, tag="yu", name="yu")
                nc.vector.tensor_copy(yu_a, ya[0:65, :])
                yu_b = yup.tile([65, 512], f32, tag="yu", name="yu")
                nc.vector.tensor_copy(yu_b, yb[0:65, :])
                state["t"] = (yu_a, yu_b)
            return state, run

        # norm part B (delayed ~ND slots): reciprocal via Ln + Exp(-x) on ACT
        # (denominator rows staged at partitions 0/32 of persistent dv2),
        # broadcast across partitions via the K=33 sel matmul, normalize into
        # yt, then fire the pair AllGather for this chunk.
        def make_norm_b(p, j, state):
            def run():
                yu_a, yu_b = state["t"]
                cs = slice(512 * j, 512 * j + 512)
                nc.vector.tensor_copy(dv2[0:1, :], yu_a[64:65, :])
                nc.vector.tensor_copy(dv2[32:33, :], yu_b[64:65, :])
                dln = dvp.tile([33, 512], f32, tag="dln", name="dln")
                nc.scalar.activation(dln, dv2, AF.Ln)
                with nc.allow_low_precision(reason="softmax denom bf16"):
                    nc.scalar.activation(dvb, dln, AF.Exp, scale=-1.0)
                db = ypp.tile([128, 512], f32, tag="y", name="db")
                nc.tensor.matmul(
                    db, lhsT=sel_sb, rhs=dvb,
                    start=True, stop=True, skip_group_check=True,
                )
                with nc.allow_low_precision(reason="y normalize bf16"):
                    nc.vector.tensor_mul(
                        yt_sb[p][0:64, cs], yu_a[0:64, :], db[0:64, :]
                    )
                    nc.vector.tensor_mul(
                        yt_sb[p][64:128, cs], yu_b[0:64, :], db[64:128, :]
                    )
                nc.sync.dma_start(out=ag_in[p][j][:, :], in_=yt_sb[p][:, cs])
                nc.gpsimd.collective_compute(
                    "AllGather", mybir.AluOpType.bypass, replica_groups=RG,
                    ins=[ag_in[p][j].ap().opt()], outs=[ag_out[p][j].ap().opt()],
                )
                nc.scalar.dma_start(out=ytf_sb[p][:, cs], in_=ag_out[p][j][0:128, :])
                nc.scalar.dma_start(
                    out=ytf_sb[4 + p][:, cs], in_=ag_out[p][j][128:256, :]
                )
            return run

        # ---------------- global slot stream (j-major) ----------------
        chunks = [(p, j) for j in range(NJ) for p in range(NP)]
        slots = []
        base = {}
        for p, j in chunks:
            base[(p, j)] = len(slots)
            for i in range(4 * j + 4):
                slots.append((p, j, i))
        n_s = len(slots)
        base_g = [base[(0, j)] for j in range(NJ)]         # group starts
        glen = [4 * (4 * j + 4) for j in range(NJ)]        # group lengths

        # fill queue: (deadline_slot, seq, closure)
        fills = []
        seq = [0]

        def push_fill(deadline, run):
            heapq.heappush(fills, (deadline, seq[0], run))
            seq[0] += 1

        # v tiles 4..15: shortly before their group (late enough that the
        # xt n-block DMA has landed, early enough for chunk (0, gi) slot i)
        for i in range(4, 16):
            gi = i // 4
            push_fill(base_g[gi] - 10 + 2 * (i % 4), v_ttile(i))
        # qk(p, *, n): group 0 pairs 1..3 just before their chunks; groups
        # n>=1 in the 8 slots before group n starts (xt n-block must be in)
        for p in range(1, NP):
            push_fill(4 * p - 3, qk_mtile(p, 0, 0))
            push_fill(4 * p - 2, qk_mtile(p, 1, 0))
        for n in range(1, NJ):
            for p in range(NP):
                push_fill(base_g[n] - 8 + 2 * p, qk_mtile(p, 0, n))
                push_fill(base_g[n] - 7 + 2 * p, qk_mtile(p, 1, n))

        # pre-stream: q/k (p0, n0) + v0..3
        qk_mtile(0, 0, 0)()
        qk_mtile(0, 1, 0)()
        for i in range(4):
            v_ttile(i)()

        ypss = {}           # chunkidx -> (ya, yb)
        pts = {}            # slot t -> P tile
        normq = []          # [slots_remaining, norm_b closure]
        chunk_of_slot = {}
        for idx, (p, j) in enumerate(chunks):
            for i in range(4 * j + 4):
                chunk_of_slot[len(chunk_of_slot)] = idx

        for t in range(n_s + LAG):
            # trailing PV + immediate norm-a for finished chunks first so
            # their releases precede this slot's fills/S on the engine FIFOs
            tt = t - LAG
            if tt >= 0:
                p2, j2, i2 = slots[tt]
                idx2 = chunk_of_slot[tt]
                ya, yb = ypss[idx2]
                nkt2 = 4 * j2 + 4
                emit_pv(p2, j2, i2, pts.pop(tt), ya, yb, nkt2)
                if i2 == nkt2 - 1:
                    state, norm_a = make_norm_a(p2, j2, ya, yb)
                    norm_a()
                    normq.append([ND, make_norm_b(p2, j2, state)])
                    del ypss[idx2]
                    if p2 == NP - 1:
                        # group j2 complete: projection of its rows once the
                        # last AllGather has landed (~ND slots + ~4us CC after
                        # now); spread the 4 m-tiles into group j2+1.  j=3
                        # drains post-stream.
                        for k in range(4):
                            if j2 < NJ - 1:
                                dl = max(
                                    t + ND + 5 + 2 * k,
                                    base_g[j2 + 1] + (glen[j2 + 1] * (3 + 2 * k)) // 24,
                                )
                            else:
                                dl = n_s + LAG + 1 + k
                            push_fill(dl, proj_m(4 * j2 + k))
            # delayed norm-b closures count down in slots
            for ent in normq:
                ent[0] -= 1
            while normq and normq[0][0] <= 0:
                normq.pop(0)[1]()
            # emit due fills
            while fills and fills[0][0] <= t:
                heapq.heappop(fills)[2]()
            if t < n_s:
                p, j, i = slots[t]
                if i == 0:
                    ypss[chunk_of_slot[t]] = (
                        ypp.tile([128, 512], f32, tag="y", name="ya"),
                        ypp.tile([128, 512], f32, tag="y", name="yb"),
                    )
                pts[t] = emit_s(p, j, i)

        # drain: remaining norm-b and fills (proj of last q-chunk)
        for ent in normq:
            ent[1]()
        while fills:
            heapq.heappop(fills)[2]()

    if split_waits:
        _split_multiwait(nc)
    return nc


def _get_nc():
    if "nc" not in _cache:
        _cache["nc"] = _build()
    return _cache["nc"]


def _make_in_maps(x, w_attn, b_attn, w_proj, b_proj):
    bf = ml_dtypes.bfloat16
    in_maps = []
    for core in range(NCORES):
        b, g = core // 2, core % 2
        qs = slice(512 * g, 512 * g + 512)
        ks = slice(1024 + 512 * g, 1024 + 512 * g + 512)
        vs = slice(2048 + 512 * g, 2048 + 512 * g + 512)
        xt = np.ascontiguousarray(np.asarray(x[b]).T).astype(bf)
        # pair-blocked: cols [256p:256p+128]=q_p (pre-scaled), then k_p
        wq = np.asarray(w_attn[:, qs], dtype=np.float64) * 0.125
        wk = np.asarray(w_attn[:, ks], dtype=np.float64)
        wqk = np.concatenate(
            [np.concatenate([wq[:, 128 * p : 128 * p + 128],
                             wk[:, 128 * p : 128 * p + 128]], axis=1)
             for p in range(NP)], axis=1
        ).astype(bf)
        wv = np.asarray(w_attn[:, vs]).astype(bf)
        wp = np.asarray(w_proj[:, 512 * g : 512 * g + 512]).astype(bf)
        in_maps.append(dict(xt=xt, wqk=wqk, wv=wv, wp=wp))
    return in_maps


def _run(in_maps, trace=False, **kw):
    from concourse.bass_utils import run_bass_kernel_spmd

    nc = _get_nc()
    return run_bass_kernel_spmd(
        nc, in_maps, core_ids=list(range(NCORES)), trace=trace, **kw
    )


def kernel(x, w_attn, b_attn, w_proj, b_proj):
    in_maps = _make_in_maps(x, w_attn, b_attn, w_proj, b_proj)
    res = _run(in_maps, trace=False)
    y = np.zeros((B, T, C), np.float32)
    for core in range(NCORES):
        b, g = core // 2, core % 2
        y[b][:, 512 * g : 512 * g + 512] = np.asarray(res.results[core]["out"])
    return y


# revision 33
# speedup vs baseline: 1.4197x; 1.0085x over previous
"""Causal self-attention (B=4, T=2048, C=1024, 16 heads) on 8 TRN2 NeuronCores.

Sharding: core = 2*b + g  (b = batch 0..3, g = head-group 0..1, 8 heads each).
Each core computes QKV for its 8 heads, causal attention, then the columns
out[:, 512g:512g+512] of the output projection.  The projection needs the
full y = concat(heads), so the two cores of each batch exchange their yT
chunks with a pair AllGather (bf16, partition-axis concat -> absolute head
order identical on both ranks -> uniform SPMD graph).

v4 schedule: single global PE stream, J-MAJOR (q-chunk outer, head-pair
inner).  Each q-chunk j's four pair-chunks finish back to back, their
AllGathers fire immediately (spread uniformly over the run instead of
clustering at the end), and the projection of rows [512j, 512j+512) runs
during group j+1 -- only the last group's projection remains after the
slot stream, so the tail is one AllGather + 4 m-tiles.

Attention slots (S-pair matmul -> one exp on ACT -> PV pair, LAG=2) are
interleaved with deadline-paced fills (QKV m-tiles, V t-tiles, projection
chains).  ACT does ONLY exp (one act per slot, 2-segment strided AP on
diagonal tiles).  The causal mask is a DVE multiply with a precomputed 0/1
triangle after exp.  The softmax reciprocal runs on DVE
(reciprocal_approx_fast, fp32, ~18 bits) and is broadcast across partitions
by a K=33 PE matmul against a 0/1 selection matrix.

Input DMA is priority-ordered across 4 queues (sync/vector carry wv then
wqk split even/odd then wp; gpsimd/scalar carry xt even/odd kc) so the
first V/QK tiles land as early as possible after the ~11us DMA-init dead
time.

Layouts (bf16 on device except f32 psum/yu; out f32):
  xt  = X[b]^T              [1024, 2048]   (host pre-transpose)
  Q^T, K^T                  [512, 2048]    hd on partitions (4 tiles, head pair each)
  V natural + ones column   [128, 8, 65] per k-tile (PV -> y^T and softmax denom)
  S^T = K^T.T @ Q^T         [k=128, q<=512] per head, pair packed via row groups
  P = exp(S^T) (bf16)       diag blocks masked by 0/1 triangle multiply
  y^T[65, q] = V_aug^T @ P  row 64 = softmax denominator
  proj (per q-chunk j, after pair AllGather of all 4 pairs):
    out[128m:128m+128, :] = sum_c ytf[c][:, m-block]^T @ wp[c]   (psum chain)
"""

import numpy as np
import ml_dtypes

B, T, C = 4, 2048, 1024
H, HD = 16, 64
NCORES = 8
HL = 8            # local heads per core
NP = 4            # head pairs per core
NKT = T // 128    # 16 k-tiles
NJ = 4            # q-chunks of 512
RG = [[0, 1], [2, 3], [4, 5], [6, 7]]
LAG = 2
ND = 5            # norm_b delay in slots after norm_a

_cache = {}


def _split_multiwait(nc):
    """walrus in this image accepts only ONE embedded wait per instruction;
    split extras into single-wait NoOps on the same engine just before it."""
    import concourse.mybir as mybir

    for fn in nc.m.functions:
        for blk in fn.blocks:
            new = []
            for inst in blk.instructions:
                si = getattr(inst, "sync_info", None)
                if si is not None and si.on_wait is not None and len(si.on_wait) > 1:
                    waits = list(si.on_wait)
                    for k, w in enumerate(waits[:-1]):
                        nop = mybir.InstNoOp(name=f"{inst.name}-w{k}")
                        nop.engine = inst.engine
                        nop.sync_info = mybir.SyncInfo(on_wait=[w], on_update=[])
                        new.append(nop)
                    inst.sync_info = mybir.SyncInfo(
                        on_wait=[waits[-1]], on_update=list(si.on_update or [])
                    )
                new.append(inst)
            blk.instructions = new


def _build(split_waits=True):
    import heapq
    import concourse.bass as bass
    import concourse.mybir as mybir
    import concourse.tile as tile
    from contextlib import ExitStack

    bf16 = mybir.dt.bfloat16
    f32 = mybir.dt.float32
    AF = mybir.ActivationFunctionType
    nc = bass.Bass(num_devices=NCORES)

    xt = nc.declare_dram_parameter("xt", [C, T], bf16, isOutput=False)
    wqk = nc.declare_dram_parameter("wqk", [C, 1024], bf16, isOutput=False)
    wv = nc.declare_dram_parameter("wv", [C, 512], bf16, isOutput=False)
    wp = nc.declare_dram_parameter("wp", [C, 512], bf16, isOutput=False)
    out = nc.declare_dram_parameter("out", [T, 512], f32, isOutput=True)

    ag_in = [[nc.dram_tensor(f"ag_in{p}_{j}", [128, 512], bf16) for j in range(NJ)]
             for p in range(NP)]
    ag_out = [[nc.dram_tensor(f"ag_out{p}_{j}", [256, 512], bf16) for j in range(NJ)]
              for p in range(NP)]
    # tiny warmup collective: absorbs the ~10us first-collective init cost
    # during the DMA-init dead time at kernel start (data unused)
    agw_in = nc.dram_tensor("agw_in", [128, 2], bf16)
    agw_out = nc.dram_tensor("agw_out", [256, 2], bf16)

    with ExitStack() as ctx:
        tc = ctx.enter_context(tile.TileContext(nc))
        pers = ctx.enter_context(tc.tile_pool(name="pers", bufs=1))
        pp = ctx.enter_context(tc.tile_pool(name="pp", bufs=4))
        yup = ctx.enter_context(tc.tile_pool(name="yup", bufs=4))
        dvp = ctx.enter_context(tc.tile_pool(name="dvp", bufs=2))
        osp = ctx.enter_context(tc.tile_pool(name="osp", bufs=2))
        spp = ctx.enter_context(tc.tile_pool(name="spp", bufs=2, space="PSUM"))
        ypp = ctx.enter_context(tc.tile_pool(name="ypp", bufs=4, space="PSUM"))

        # ---------------- persistent tiles ----------------
        xt_sb = [pers.tile([128, T], bf16, tag=f"xt{i}", name=f"xt{i}") for i in range(8)]
        wqk_sb = [pers.tile([128, 1024], bf16, tag=f"wqk{i}", name=f"wqk{i}") for i in range(8)]
        wv_sb = [pers.tile([128, 512], bf16, tag=f"wv{i}", name=f"wv{i}") for i in range(8)]
        wp_sb = [pers.tile([128, 512], bf16, tag=f"wp{i}", name=f"wp{i}") for i in range(8)]
        qt_sb = [pers.tile([128, T], bf16, tag=f"qt{p}", name=f"qt{p}") for p in range(NP)]
        kt_sb = [pers.tile([128, T], bf16, tag=f"kt{p}", name=f"kt{p}") for p in range(NP)]
        v_sb = [pers.tile([128, HL, 65], bf16, tag=f"v{i}", name=f"v{i}") for i in range(NKT)]
        yt_sb = [pers.tile([128, T], bf16, tag=f"yt{p}", name=f"yt{p}") for p in range(NP)]
        ytf_sb = [pers.tile([128, T], bf16, tag=f"ytf{c}", name=f"ytf{c}") for c in range(8)]
        # 0/1 causal keep-mask for the two heads' diagonal 128x128 blocks
        mask2 = pers.tile([128, 2, 128], bf16, tag="mask2", name="mask2")
        # selection matrix for the single K=33 denominator-broadcast matmul:
        # db[m, q] = sum_r sel[r, m] * dvb[r, q]; row 0 selects head a
        # (m < 64), row 32 head b (m >= 64); rows 1..31 of both sel and dvb
        # are zero so they contribute nothing.
        sel_sb = pers.tile([33, 128], bf16, tag="sel", name="sel")
        dvb = pers.tile([33, 512], bf16, tag="dvb", name="dvb")
        # persistent Ln input; rows 1..31 stay 1.0 forever -> Ln = 0 ->
        # Exp(-0) = 1.0 (finite) in dvb's unused rows, zeroed by sel
        dv2 = pers.tile([33, 512], f32, tag="dv2", name="dv2")

        # ---------------- input loads (priority-ordered, 3 queues) ----------
        # sync: wqk pair-major (pair 0 first -> first QK m-tiles asap), wp.
        # scalar: wv odd rows, xt odd rows n-major.  gpsimd: wv even, xt even.
        # wqk host layout is PAIR-BLOCKED: cols [256p:256p+128]=q_p,
        # [256p+128:256p+256]=k_p, so per-pair loads are contiguous.
        for p in range(NP):
            for i in range(8):
                nc.sync.dma_start(
                    out=wqk_sb[i][:, 256 * p : 256 * p + 256],
                    in_=wqk[128 * i : 128 * i + 128, 256 * p : 256 * p + 256],
                )
        for i in range(8):
            eng = nc.gpsimd if i % 2 == 0 else nc.scalar
            eng.dma_start(out=wv_sb[i], in_=wv[128 * i : 128 * i + 128, :])
        for n in range(NJ):
            for i in range(8):
                eng = nc.gpsimd if i % 2 == 0 else nc.scalar
                eng.dma_start(
                    out=xt_sb[i][:, 512 * n : 512 * n + 512],
                    in_=xt[128 * i : 128 * i + 128, 512 * n : 512 * n + 512],
                )
        for i in range(8):
            nc.sync.dma_start(out=wp_sb[i], in_=wp[128 * i : 128 * i + 128, :])

        # warm up the collectives path while input DMA streams
        nc.gpsimd.collective_compute(
            "AllGather", mybir.AluOpType.bypass, replica_groups=RG,
            ins=[agw_in.ap().opt()], outs=[agw_out.ap().opt()],
        )

        # masks (after the DMA issues so they don't delay the loads):
        # mask2[r, s, q'] = 1 iff q' >= r (keep), else 0
        nc.gpsimd.memset(mask2, 1.0)
        nc.gpsimd.affine_select(
            out=mask2, in_=mask2, compare_op=mybir.AluOpType.is_ge, fill=0.0,
            base=0, pattern=[[0, 2], [1, 128]], channel_multiplier=-1,
        )
        nc.vector.memset(sel_sb, 0.0)
        nc.vector.memset(sel_sb[0:1, 0:64], 1.0)
        nc.vector.memset(sel_sb[32:33, 64:128], 1.0)
        nc.vector.memset(dvb, 0.0)
        nc.vector.memset(dv2, 1.0)

        # ---------------- fill closures ----------------
        def v_ttile(i):
            def run():
                ps = ypp.tile([128, 512], f32, tag="y", name="psv")
                for kc in range(8):
                    nc.tensor.matmul(
                        ps,
                        lhsT=xt_sb[kc][:, 128 * i : 128 * i + 128],
                        rhs=wv_sb[kc],
                        start=(kc == 0), stop=(kc == 7),
                    )
                nc.vector.tensor_copy(
                    out=v_sb[i][:, :, 0:64],
                    in_=ps.rearrange("p (h d) -> p h d", h=HL),
                )
                nc.vector.memset(v_sb[i][:, :, 64:65], 1.0)
            return run

        def qk_mtile(p, which, n):
            col0 = 256 * p + 128 * which
            dst = kt_sb[p] if which == 1 else qt_sb[p]

            def run():
                ps = ypp.tile([128, 512], f32, tag="y", name="psqk")
                for kc in range(8):
                    nc.tensor.matmul(
                        ps,
                        lhsT=wqk_sb[kc][:, col0 : col0 + 128],
                        rhs=xt_sb[kc][:, 512 * n : 512 * n + 512],
                        start=(kc == 0), stop=(kc == 7),
                    )
                nc.vector.tensor_copy(
                    out=dst[:, 512 * n : 512 * n + 512], in_=ps,
                )
            return run

        def proj_m(m):
            def run():
                po = ypp.tile([128, 512], f32, tag="y", name="po")
                for c in range(8):
                    nc.tensor.matmul(
                        po,
                        lhsT=ytf_sb[c][:, 128 * m : 128 * m + 128],
                        rhs=wp_sb[c],
                        start=(c == 0), stop=(c == 7),
                    )
                os = osp.tile([128, 512], f32, tag="os", name="os")
                nc.vector.tensor_copy(os, po)
                nc.gpsimd.dma_start(out=out[128 * m : 128 * m + 128, :], in_=os)
            return run

        # ---------------- attention slot pieces ----------------
        def emit_s(p, j, i):
            """S^T pair matmuls (row-group packed) + exp -> P tile; diag
            blocks masked by 0/1 triangle multiply on DVE."""
            dlt = 128 * i - 512 * j
            de = max(0, dlt)
            diag = dlt >= 0
            sp = spp.tile([128, 1024], f32, tag="sp", name="sp")
            for h in range(2):  # head pair (K=64 each, packed on row groups)
                r0, c0 = 64 * h, 512 * h
                nc.tensor.matmul(
                    sp[:, c0 + de : c0 + 512],
                    lhsT=kt_sb[p][r0 : r0 + 64, 128 * i : 128 * i + 128],
                    rhs=qt_sb[p][r0 : r0 + 64, 512 * j + de : 512 * j + 512],
                    start=True, stop=True, skip_group_check=True,
                )
            pt = pp.tile([128, 1024], bf16, tag="pt", name="pt")
            if de > 0:
                sp3 = sp.rearrange("p (s q) -> p s q", s=2)[:, :, de:512]
                pt3 = pt.rearrange("p (s q) -> p s q", s=2)[:, :, de:512]
                nc.scalar.activation(pt3, sp3, AF.Exp)
            else:
                nc.scalar.activation(pt, sp, AF.Exp)
            if diag:
                ptm = pt.rearrange("p (s q) -> p s q", s=2)[:, :, de : de + 128]
                nc.vector.tensor_mul(ptm, ptm, mask2)
            return pt

        def emit_pv(p, j, i, pt, ya, yb, nkt):
            dlt = 128 * i - 512 * j
            de = max(0, dlt)
            nc.tensor.matmul(
                ya[0:65, de:512],
                lhsT=v_sb[i][:, 2 * p, :], rhs=pt[:, de:512],
                start=(i == 0), stop=(i == nkt - 1), skip_group_check=True,
            )
            nc.tensor.matmul(
                yb[0:65, de:512],
                lhsT=v_sb[i][:, 2 * p + 1, :], rhs=pt[:, 512 + de : 1024],
                start=(i == 0), stop=(i == nkt - 1), skip_group_check=True,
            )

        # norm part A (immediate after last PV): evacuate y to SBUF
        def make_norm_a(p, j, ya, yb):
            state = {}

            def run():
                yu_a = yup.tile([65, 512], f32, tag="yu", name="yu")
                nc.vector.tensor_copy(yu_a, ya[0:65, :])
                yu_b = yup.tile([65, 512], f32, tag="yu", name="yu")
                nc.vector.tensor_copy(yu_b, yb[0:65, :])
                state["t"] = (yu_a, yu_b)
            return state, run

        # norm part B (delayed ~ND slots): reciprocal via Ln + Exp(-x) on ACT
        # (denominator rows staged at partitions 0/32 of persistent dv2),
        # broadcast across partitions via the K=33 sel matmul, normalize into
        # yt, then fire the pair AllGather for this chunk.
        def make_norm_b(p, j, state):
            def run():
                yu_a, yu_b = state["t"]
                cs = slice(512 * j, 512 * j + 512)
                nc.vector.tensor_copy(dv2[0:1, :], yu_a[64:65, :])
                nc.vector.tensor_copy(dv2[32:33, :], yu_b[64:65, :])
                dln = dvp.tile([33, 512], f32, tag="dln", name="dln")
                nc.scalar.activation(dln, dv2, AF.Ln)
                with nc.allow_low_precision(reason="softmax denom bf16"):
                    nc.scalar.activation(dvb, dln, AF.Exp, scale=-1.0)
                db = ypp.tile([128, 512], f32, tag="y", name="db")
                nc.tensor.matmul(
                    db, lhsT=sel_sb, rhs=dvb,
                    start=True, stop=True, skip_group_check=True,
                )
                with nc.allow_low_precision(reason="y normalize bf16"):
                    nc.vector.tensor_mul(
                        yt_sb[p][0:64, cs], yu_a[0:64, :], db[0:64, :]
                    )
                    nc.vector.tensor_mul(
                        yt_sb[p][64:128, cs], yu_b[0:64, :], db[64:128, :]
                    )
                nc.sync.dma_start(out=ag_in[p][j][:, :], in_=yt_sb[p][:, cs])
                nc.gpsimd.collective_compute(
                    "AllGather", mybir.AluOpType.bypass, replica_groups=RG,
                    ins=[ag_in[p][j].ap().opt()], outs=[ag_out[p][j].ap().opt()],
                )
                nc.scalar.dma_start(out=ytf_sb[p][:, cs], in_=ag_out[p][j][0:128, :])
                nc.scalar.dma_start(
                    out=ytf_sb[4 + p][:, cs], in_=ag_out[p][j][128:256, :]
                )
            return run

        # ---------------- global slot stream (j-major) ----------------
        chunks = [(p, j) for j in range(NJ) for p in range(NP)]
        slots = []
        base = {}
        for p, j in chunks:
            base[(p, j)] = len(slots)
            for i in range(4 * j + 4):
                slots.append((p, j, i))
        n_s = len(slots)
        base_g = [base[(0, j)] for j in range(NJ)]         # group starts
        glen = [4 * (4 * j + 4) for j in range(NJ)]        # group lengths

        # fill queue: (deadline_slot, seq, closure)
        fills = []
        seq = [0]

        def push_fill(deadline, run):
            heapq.heappush(fills, (deadline, seq[0], run))
            seq[0] += 1

        # v tiles 4..15: late-middle of the previous group (xt n-block landed)
        for i in range(4, 16):
            gi = i // 4
            w = glen[gi - 1] // 3
            push_fill(base_g[gi] - w + (w * (i % 4)) // 4, v_ttile(i))
        # qk(p, *, n): group 0 pairs 1..3 just before their chunks; groups
        # n>=1 spread across the second half of group n-1 (xt n-block must
        # have landed by then)
        for p in range(1, NP):
            push_fill(4 * p - 3, qk_mtile(p, 0, 0))
            push_fill(4 * p - 2, qk_mtile(p, 1, 0))
        for n in range(1, NJ):
            h = glen[n - 1] // 2
            for p in range(NP):
                push_fill(base_g[n] - h + (h * (2 * p)) // 8, qk_mtile(p, 0, n))
                push_fill(base_g[n] - h + (h * (2 * p + 1)) // 8, qk_mtile(p, 1, n))

        # pre-stream: v0 + q/k (p0, n0) + v1..3
        v_ttile(0)()
        qk_mtile(0, 0, 0)()
        qk_mtile(0, 1, 0)()
        for i in range(1, 4):
            v_ttile(i)()

        ypss = {}           # chunkidx -> (ya, yb)
        pts = {}            # slot t -> P tile
        normq = []          # [slots_remaining, norm_b closure]
        chunk_of_slot = {}
        for idx, (p, j) in enumerate(chunks):
            for i in range(4 * j + 4):
                chunk_of_slot[len(chunk_of_slot)] = idx

        for t in range(n_s + LAG):
            # trailing PV + immediate norm-a for finished chunks first so
            # their releases precede this slot's fills/S on the engine FIFOs
            tt = t - LAG
            if tt >= 0:
                p2, j2, i2 = slots[tt]
                idx2 = chunk_of_slot[tt]
                ya, yb = ypss[idx2]
                nkt2 = 4 * j2 + 4
                emit_pv(p2, j2, i2, pts.pop(tt), ya, yb, nkt2)
                if i2 == nkt2 - 1:
                    state, norm_a = make_norm_a(p2, j2, ya, yb)
                    norm_a()
                    normq.append([ND, make_norm_b(p2, j2, state)])
                    del ypss[idx2]
                    if p2 == NP - 1:
                        # group j2 complete: projection of its rows once the
                        # last AllGather has landed.  The last norm_b emits at
                        # ~t+ND and the AG chain takes ~6us (~8 slots), so
                        # deadline ~t+ND+12, spread 3 apart.  j=3 drains
                        # post-stream.
                        for k in range(4):
                            if j2 < NJ - 1:
                                dl = t + ND + 12 + 3 * k
                            else:
                                dl = n_s + LAG + 1 + k
                            push_fill(dl, proj_m(4 * j2 + k))
            # delayed norm-b closures count down in slots
            for ent in normq:
                ent[0] -= 1
            while normq and normq[0][0] <= 0:
                normq.pop(0)[1]()
            # emit due fills
            while fills and fills[0][0] <= t:
                heapq.heappop(fills)[2]()
            if t < n_s:
                p, j, i = slots[t]
                if i == 0:
                    ypss[chunk_of_slot[t]] = (
                        ypp.tile([128, 512], f32, tag="y", name="ya"),
                        ypp.tile([128, 512], f32, tag="y", name="yb"),
                    )
                pts[t] = emit_s(p, j, i)

        # drain: remaining norm-b and fills (proj of last q-chunk)
        for ent in normq:
            ent[1]()
        while fills:
            heapq.heappop(fills)[2]()

    if split_waits:
        _split_multiwait(nc)
    return nc


def _get_nc():
    if "nc" not in _cache:
        _cache["nc"] = _build()
    return _cache["nc"]


def _make_in_maps(x, w_attn, b_attn, w_proj, b_proj):
    bf = ml_dtypes.bfloat16
    in_maps = []
    for core in range(NCORES):
        b, g = core // 2, core % 2
        qs = slice(512 * g, 512 * g + 512)
        ks = slice(1024 + 512 * g, 1024 + 512 * g + 512)
        vs = slice(2048 + 512 * g, 2048 + 512 * g + 512)
        xt = np.ascontiguousarray(np.asarray(x[b]).T).astype(bf)
        # pair-blocked: cols [256p:256p+128]=q_p (pre-scaled), then k_p
        wq = np.asarray(w_attn[:, qs], dtype=np.float64) * 0.125
        wk = np.asarray(w_attn[:, ks], dtype=np.float64)
        wqk = np.concatenate(
            [np.concatenate([wq[:, 128 * p : 128 * p + 128],
                             wk[:, 128 * p : 128 * p + 128]], axis=1)
             for p in range(NP)], axis=1
        ).astype(bf)
        wv = np.asarray(w_attn[:, vs]).astype(bf)
        wp = np.asarray(w_proj[:, 512 * g : 512 * g + 512]).astype(bf)
        in_maps.append(dict(xt=xt, wqk=wqk, wv=wv, wp=wp))
    return in_maps


def _run(in_maps, trace=False, **kw):
    from concourse.bass_utils import run_bass_kernel_spmd

    nc = _get_nc()
    return run_bass_kernel_spmd(
        nc, in_maps, core_ids=list(range(NCORES)), trace=trace, **kw
    )


def kernel(x, w_attn, b_attn, w_proj, b_proj):
    in_maps = _make_in_maps(x, w_attn, b_attn, w_proj, b_proj)
    res = _run(in_maps, trace=False)
    y = np.zeros((B, T, C), np.float32)
    for core in range(NCORES):
        b, g = core // 2, core % 2
        y[b][:, 512 * g : 512 * g + 512] = np.asarray(res.results[core]["out"])
    return y
